# revision 15
# baseline (speedup 1.0000x reference)
"""Multi-head self-attention on 8 Trainium2 NeuronCores.

Problem: x(2,2048,1024), 16 heads of 64, fp32 reference. Sharding: batch (2) x
head-groups (4 groups of 4 heads). Each core computes Q/K/V projections for its
256 head-dims, attention for its 4 heads, and a partial out-projection (its 256
rows of Wo). Host sums the 4 group-partials per batch (the tensor-parallel
all-reduce) and adds bo.

Kernel layout (per core), v4 (fp16 matmuls + engine-split exp):
  All matmul operands fp16 (PSUM accumulation fp32). fp8 was tried and is
  numerically dead here: ctx is a softmax-weighted mean of ~uncorrelated v's,
  so |ctx| ~ |v|/sqrt(n_eff) and upstream quantization noise passes through
  at FULL relative strength (measured: e5m2 attn alone 2.9%, e4m3 V 1.5%,
  fp8 x/Wqk 3.3-3.6% vs the 2% budget).
  xT [KC,128,SC,512] fp16 sc-major so projections chase the DMA stream.
  QT/KT [256,2048] (head-dims on partitions, head pair per 128-tile),
  V natural [2048,256] interleaved with a ones column per head (V_aug).
  scoresT[t,s] per head-PAIR: two K=64 matmuls in disjoint PE row halves
  (concurrent row-tiled execution), sharing one 2-bank PSUM slot.
  exp(x/8 - 1.5) engine-split per [128,1024] scores tile: ScalarE activation
  Exp for cols [0:EW); DVE Schraudolph for [EW:1024): bits16 =
  rne(184.67*x + 13144 + sig) saturating-uint16, bitcast fp16 (~2% rms on
  ~44% of weights -> ~0.7% output err; constant e^-1.5 cancels in softmax).
  attnT fp16; ctxT_aug[65,s] = V_aug^T @ attnT (row 64 = denominator).
  sc-major tick pipeline: attnV(tick-1) interleaves into scores(tick).
  Normalize via DVE reciprocal + Pool broadcast + fp16 multiply.
  out partial = ctx @ Wo fp16; PSUM evacuated fp16 split ScalarE/DVE
  (halves the out DMA).
"""

import sys

sys.path.insert(0, "/opt/trn_rl_repo")

import numpy as np

import concourse.bacc as bacc
import concourse.mybir as mybir
import concourse.tile as tile
from concourse import bass_utils

N_CORES = 8
B, S, D = 2, 2048, 1024
H_LOC = 4          # heads per core
DH = 64            # head dim
DG = H_LOC * DH    # 256 group dims per core
KC = D // 128      # 8 contraction chunks over D
ST = S // 128      # 16 s/t tiles
SC = S // 512      # 4 512-wide s chunks
MT = DG // 128     # 2 m-tiles of group dims

EW = 576           # exp cols on ScalarE per [128,1024] tile; DVE does the rest
# fp16 Schraudolph: bits = rne(M*raw + B), saturating-uint16, bitcast fp16.
# exp(0.125*raw - 1.5) with sig calibrated offline; e^-1.5 cancels in softmax.
SCH_M = 1024.0 * 1.4427 * 0.125            # 184.67
SCH_B = 1024.0 * (15.0 - 1.4427 * 1.5) - 30.7  # sig=-30.7 (rms-optimal)

F32 = mybir.dt.float32
F16 = mybir.dt.float16
U16 = mybir.dt.uint16


def _build_program(reps=1, num_devices=N_CORES, phases=("proj", "attn", "out")):
    nc = bacc.Bacc("TRN2", target_bir_lowering=False, debug=False,
                   num_devices=num_devices)

    xT_d = nc.dram_tensor("xT", [KC, 128, S], F16, kind="ExternalInput")
    wq_d = nc.dram_tensor("wq", [KC, 128, DG], F16, kind="ExternalInput")
    wk_d = nc.dram_tensor("wk", [KC, 128, DG], F16, kind="ExternalInput")
    wv_d = nc.dram_tensor("wv", [KC, 128, DG], F16, kind="ExternalInput")
    bqk_d = nc.dram_tensor("bqk", [MT, 128, 2], F32, kind="ExternalInput")
    bv_d = nc.dram_tensor("bv", [1, DG], F32, kind="ExternalInput")
    wo_d = nc.dram_tensor("wo", [MT, 128, D], F16, kind="ExternalInput")
    out_d = nc.dram_tensor("out", [S, D], F16, kind="ExternalOutput")

    with tile.TileContext(nc) as tc:
      for _rep in range(reps):
        with (
            tc.tile_pool(name="wpool", bufs=1) as wpool,
            tc.tile_pool(name="mpool", bufs=1) as mpool,
            tc.tile_pool(name="psum", bufs=4, space="PSUM") as pp,
        ):
            # ---- weights / biases (batched DMAs: one per tensor) ----
            wq_t = wpool.tile([128, KC, DG], F16)
            wk_t = wpool.tile([128, KC, DG], F16)
            wv_t = wpool.tile([128, KC, DG], F16)
            wo_t = wpool.tile([128, MT, D], F16)
            bqk_t = wpool.tile([128, MT, 2], F32)
            bv_row = wpool.tile([1, DG], F32)
            bv_bc = wpool.tile([128, DG], F32)
            nbias_t = wpool.tile([128, 1], F32)   # exp bias (-1.5) for ScalarE
            warm_t = wpool.tile([128, 512], F16)
            nc.sync.dma_start(
                wq_t[:], wq_d.ap().rearrange("k p g -> p k g"))
            nc.sync.dma_start(
                wk_t[:], wk_d.ap().rearrange("k p g -> p k g"))
            nc.sync.dma_start(
                wv_t[:], wv_d.ap().rearrange("k p g -> p k g"))
            nc.sync.dma_start(
                wo_t[:], wo_d.ap().rearrange("m p d -> p m d"))
            nc.sync.dma_start(
                bqk_t[:], bqk_d.ap().rearrange("m p c -> p m c"))
            nc.sync.dma_start(bv_row[:], bv_d.ap())
            nc.gpsimd.partition_broadcast(bv_bc[:], bv_row[:])
            nc.gpsimd.memset(nbias_t[:], -1.5)
            nc.gpsimd.memset(warm_t[:], 0.0)
            # PE warmup: keep the HAM activity window busy during input DMA
            ps_w = pp.tile([128, 1024], F32, tag="ps", name="ps_w")
            for _w in range(24):
                nc.tensor.matmul(ps_w[:, 0:512], warm_t[:, 0:128],
                                 warm_t[:], start=(_w == 0), stop=(_w == 23))

            # ---- persistent intermediates ----
            qT_t = mpool.tile([128, MT, S], F16)    # [dg_row, mt, s]
            kT_t = mpool.tile([128, MT, S], F16)
            vaug = mpool.tile([128, ST, H_LOC, DH + 1], F16)
            ctxN = mpool.tile([128, MT, S], F16)    # normalized ctx^T
            nc.gpsimd.memset(vaug[:], 1.0)

            # ---- projections (k-outer for m=0 so PE chases the xT DMA) ----
            with tc.tile_pool(name="xpool", bufs=1) as xpool:
                xT_t = xpool.tile([128, KC, S], F16)
                for k in range(KC):
                    nc.sync.dma_start(xT_t[:, k, :], xT_d.ap()[k])

                for m in (range(MT) if "proj" in phases else []):
                    if m == 0:
                        # k-outer, 4 slots: PE starts on xT chunk 0 while the
                        # rest stream in
                        ps_qks = [
                            pp.tile([128, 1024], F32, tag="ps", name=f"ps_qk{sc}")
                            for sc in range(SC)
                        ]
                        for k in range(KC):
                            for sc in range(SC):
                                sl = slice(sc * 512, sc * 512 + 512)
                                nc.tensor.matmul(
                                    ps_qks[sc][:, 0:512],
                                    wq_t[:, k, 0:128],
                                    xT_t[:, k, sl],
                                    start=(k == 0), stop=(k == KC - 1))
                                nc.tensor.matmul(
                                    ps_qks[sc][:, 512:1024],
                                    wk_t[:, k, 0:128],
                                    xT_t[:, k, sl],
                                    start=(k == 0), stop=(k == KC - 1))
                        for sc in range(SC):
                            sl = slice(sc * 512, sc * 512 + 512)
                            nc.vector.tensor_scalar_add(
                                qT_t[:, m, sl], ps_qks[sc][:, 0:512],
                                bqk_t[:, m, 0:1])
                            nc.vector.tensor_scalar_add(
                                kT_t[:, m, sl], ps_qks[sc][:, 512:1024],
                                bqk_t[:, m, 1:2])
                    else:
                        # sc-outer, 1 slot at a time: frees PSUM early
                        for sc in range(SC):
                            sl = slice(sc * 512, sc * 512 + 512)
                            ps_qk = pp.tile([128, 1024], F32, tag="ps",
                                            name="ps_qk")
                            for k in range(KC):
                                nc.tensor.matmul(
                                    ps_qk[:, 0:512],
                                    wq_t[:, k, m * 128 : m * 128 + 128],
                                    xT_t[:, k, sl],
                                    start=(k == 0), stop=(k == KC - 1))
                                nc.tensor.matmul(
                                    ps_qk[:, 512:1024],
                                    wk_t[:, k, m * 128 : m * 128 + 128],
                                    xT_t[:, k, sl],
                                    start=(k == 0), stop=(k == KC - 1))
                            nc.vector.tensor_scalar_add(
                                qT_t[:, m, sl], ps_qk[:, 0:512], bqk_t[:, m, 0:1])
                            nc.vector.tensor_scalar_add(
                                kT_t[:, m, sl], ps_qk[:, 512:1024], bqk_t[:, m, 1:2])

                # V natural [t, dg], ones column per head
                for st in (range(ST) if "proj" in phases else []):
                    ps_v = pp.tile([128, 1024], F32, tag="ps", name="ps_v")
                    tsl = slice(st * 128, st * 128 + 128)
                    for k in range(KC):
                        nc.tensor.matmul(
                            ps_v[:, 0:DG],
                            xT_t[:, k, tsl],
                            wv_t[:, k, :],
                            start=(k == 0), stop=(k == KC - 1))
                    nc.vector.tensor_add(
                        vaug[:, st, :, 0:DH],
                        ps_v[:, 0:DG].rearrange("p (h d) -> p h d", h=H_LOC),
                        bv_bc[:].rearrange("p (h d) -> p h d", h=H_LOC),
                    )

            # ---- attention: flat (pair, sc) pipeline, V one tick behind ----
            with tc.tile_pool(name="apool", bufs=1) as apool:
              if "attn" in phases:
                ctxu = {
                    (pair, i): apool.tile([65, S], F16, tag=f"ctxu{pair}{i}",
                                          bufs=1, name=f"ctxu{pair}{i}")
                    for pair in range(2) for i in range(2)
                }

                def emit_normalize_chunk(pair, sc):
                    mt = pair
                    ssl = slice(sc * 512, sc * 512 + 512)
                    for i, h in enumerate((2 * pair, 2 * pair + 1)):
                        half = h % 2
                        rs_p = apool.tile([128, 4], F16, tag="rs", bufs=2,
                                          name="rs_p")
                        nc.gpsimd.dma_start(rs_p[:], ctxu[pair, i][64:65, ssl])
                        rr_p = apool.tile([128, 4], F16, tag="rr", bufs=2,
                                          name="rr_p")
                        with nc.allow_low_precision(
                                reason="denominator ~750; fp16 recip 0.05%"):
                            nc.vector.reciprocal(rr_p[:], rs_p[:])
                        r_row = apool.tile([1, 512], F16, tag="rrow", bufs=2,
                                           name="r_row")
                        nc.gpsimd.dma_start(r_row[:], rr_p[:])
                        r_bc = apool.tile([64, 512], F16, tag="rbc", bufs=2,
                                          name="r_bc")
                        nc.gpsimd.partition_broadcast(r_bc[:], r_row[:])
                        if half == 0:
                            nc.vector.tensor_mul(
                                ctxN[0:64, mt, ssl],
                                ctxu[pair, i][0:64, ssl], r_bc[:])
                        else:
                            csh = apool.tile([64, 512], F16, tag="csh", bufs=2,
                                             name="csh")
                            nc.vector.tensor_mul(
                                csh[:], ctxu[pair, i][0:64, ssl], r_bc[:])
                            nc.gpsimd.dma_start(ctxN[64:128, mt, ssl], csh[:])

                def emit_outproj(st):
                    ps_o = pp.tile([128, 1024], F32, tag="ps", name="ps_o")
                    for m in range(MT):
                        for n in range(2):
                            nsl = slice(n * 512, n * 512 + 512)
                            nc.tensor.matmul(
                                ps_o[:, nsl],
                                ctxN[:, m, st * 128 : st * 128 + 128],
                                wo_t[:, m, nsl],
                                start=(m == 0), stop=(m == MT - 1))
                    o4 = emit_outproj.o4
                    if o4 is None:
                        o4 = apool.tile([128, 4, 1024], F16, tag="ot", bufs=2,
                                        name="o4")
                        emit_outproj.o4 = o4
                    nc.scalar.copy(o4[:, st % 4, 0:512], ps_o[:, 0:512])
                    nc.vector.tensor_copy(o4[:, st % 4, 512:1024],
                                          ps_o[:, 512:1024])
                    if st % 4 == 3:
                        st0 = st - 3
                        nc.sync.dma_start(
                            out_d.ap()[st0 * 128 : st0 * 128 + 512, :]
                            .rearrange("(f p) d -> p f d", p=128),
                            o4[:])
                        emit_outproj.o4 = None
                emit_outproj.o4 = None

                # pair-alternating order: (1,sc) completes every other tick,
                # spreading the out-projection across the phase instead of
                # bunching it into the last four ticks
                ticks = [(p, sc) for sc in range(SC) for p in range(2)]
                prev = None  # (pair, sc, tiles)
                for t in range(len(ticks) + 1):
                    cur_tick = ticks[t] if t < len(ticks) else None
                    ps_c = (pp.tile([128, 1024], F32, tag="ps", name="ps_c")
                            if prev is not None else None)
                    cur = []
                    for st in range(ST):
                        tsl = slice(st * 128, st * 128 + 128)
                        if cur_tick is not None:
                            pair, sc = cur_tick
                            ssl = slice(sc * 512, sc * 512 + 512)
                            ps_s = pp.tile([128, 1024], F32, tag="ps",
                                           name="ps_s")
                            # two K=64 matmuls in disjoint PE row halves
                            nc.tensor.matmul(
                                ps_s[:, 0:512],
                                kT_t[0:64, pair, tsl], qT_t[0:64, pair, ssl])
                            nc.tensor.matmul(
                                ps_s[:, 512:1024],
                                kT_t[64:128, pair, tsl], qT_t[64:128, pair, ssl])
                            at = apool.tile([128, 1024], F16, tag="attnT",
                                            bufs=36, name="at")
                            # engine-split exp: ScalarE [0:EW), DVE Schraudolph
                            nc.scalar.activation(
                                at[:, 0:EW], ps_s[:, 0:EW],
                                mybir.ActivationFunctionType.Exp,
                                bias=nbias_t[:], scale=0.125)
                            nc.vector.tensor_scalar(
                                at[:, EW:1024].bitcast(U16),
                                ps_s[:, EW:1024], SCH_M, SCH_B,
                                mybir.AluOpType.mult, mybir.AluOpType.add)
                            cur.append(at)
                        if prev is not None:
                            vpair, vsc, tiles = prev
                            k = st
                            for i, h in enumerate((2 * vpair, 2 * vpair + 1)):
                                csl = slice(i * 512, i * 512 + 512)
                                nc.tensor.matmul(
                                    ps_c[0:65, csl],
                                    vaug[:, k, h, :],
                                    tiles[k][:, csl],
                                    start=(k == 0), stop=(k == ST - 1))
                    if prev is not None:
                        vpair, vsc, _ = prev
                        psl = slice(vsc * 512, vsc * 512 + 512)
                        nc.scalar.copy(ctxu[vpair, 0][:, psl], ps_c[0:65, 0:512])
                        nc.vector.tensor_copy(
                            ctxu[vpair, 1][:, psl], ps_c[0:65, 512:1024])
                        emit_normalize_chunk(vpair, vsc)
                        if vpair == 1 and "out" in phases:
                            for st_o in range(vsc * 4, vsc * 4 + 4):
                                emit_outproj(st_o)
                    prev = (cur_tick[0], cur_tick[1], cur) if cur_tick else None

    nc.compile()
    return nc


_CACHE = {}


def _get_program():
    if "nc" not in _CACHE:
        _CACHE["nc"] = _build_program()
    return _CACHE["nc"]


def _shard_inputs(x, Wq, bq, Wk, bk, Wv, bv, Wo):
    xT16 = [
        np.ascontiguousarray(x[b].T).astype(np.float16).reshape(KC, 128, S)
        for b in range(B)
    ]
    in_maps = []
    for c in range(N_CORES):
        b, g = c // 4, c % 4
        gs = slice(g * DG, g * DG + DG)
        bqk = np.stack([bq[gs], bk[gs]], axis=-1)  # [DG, 2]
        in_maps.append({
            "xT": xT16[b],
            "wq": np.ascontiguousarray(Wq[:, gs]).astype(np.float16).reshape(KC, 128, DG),
            "wk": np.ascontiguousarray(Wk[:, gs]).astype(np.float16).reshape(KC, 128, DG),
            "wv": np.ascontiguousarray(Wv[:, gs]).astype(np.float16).reshape(KC, 128, DG),
            "bqk": np.ascontiguousarray(bqk).astype(np.float32).reshape(MT, 128, 2),
            "bv": np.ascontiguousarray(bv[gs]).astype(np.float32).reshape(1, DG),
            "wo": np.ascontiguousarray(Wo[gs, :]).astype(np.float16).reshape(MT, 128, D),
        })
    return in_maps


def kernel(x, Wq, bq, Wk, bk, Wv, bv, Wo, bo, _trace=False, _trace_kwargs=None):
    x = np.asarray(x, dtype=np.float32)
    Wq, bq = np.asarray(Wq, np.float32), np.asarray(bq, np.float32)
    Wk, bk = np.asarray(Wk, np.float32), np.asarray(bk, np.float32)
    Wv, bv = np.asarray(Wv, np.float32), np.asarray(bv, np.float32)
    Wo, bo = np.asarray(Wo, np.float32), np.asarray(bo, np.float32)

    nc = _get_program()
    in_maps = _shard_inputs(x, Wq, bq, Wk, bk, Wv, bv, Wo)
    kwargs = {}
    if _trace:
        kwargs["trace"] = True
        kwargs.update(_trace_kwargs or {})
    res = bass_utils.run_bass_kernel_spmd(
        nc, in_maps, core_ids=list(range(N_CORES)), **kwargs)

    out = np.zeros((B, S, D), dtype=np.float32)
    for c in range(N_CORES):
        out[c // 4] += np.asarray(res.results[c]["out"], dtype=np.float32)
    out += bo
    if _trace:
        kernel.last_result = res
    return out


# revision 16
# speedup vs baseline: 1.1124x; 1.1124x over previous
"""Multi-head self-attention on 8 Trainium2 NeuronCores.

Problem: x(2,2048,1024), 16 heads of 64, fp32 reference. Sharding: batch (2) x
head-groups (4 groups of 4 heads). Each core computes Q/K/V projections for its
256 head-dims, attention for its 4 heads, and a partial out-projection (its 256
rows of Wo). Host sums the 4 group-partials per batch (the tensor-parallel
all-reduce) and adds bo.

Kernel layout (per core), v4 (fp16 matmuls + engine-split exp):
  All matmul operands fp16 (PSUM accumulation fp32). fp8 was tried and is
  numerically dead here: ctx is a softmax-weighted mean of ~uncorrelated v's,
  so |ctx| ~ |v|/sqrt(n_eff) and upstream quantization noise passes through
  at FULL relative strength (measured: e5m2 attn alone 2.9%, e4m3 V 1.5%,
  fp8 x/Wqk 3.3-3.6% vs the 2% budget).
  xT [KC,128,SC,512] fp16 sc-major so projections chase the DMA stream.
  QT/KT [256,2048] (head-dims on partitions, head pair per 128-tile),
  V natural [2048,256] interleaved with a ones column per head (V_aug).
  scoresT[t,s] per head-PAIR: two K=64 matmuls in disjoint PE row halves
  (concurrent row-tiled execution), sharing one 2-bank PSUM slot.
  exp(x/8 - 1.5) engine-split per [128,1024] scores tile: ScalarE activation
  Exp for cols [0:EW); DVE Schraudolph for [EW:1024): bits16 =
  rne(184.67*x + 13144 + sig) saturating-uint16, bitcast fp16 (~2% rms on
  ~44% of weights -> ~0.7% output err; constant e^-1.5 cancels in softmax).
  attnT fp16; ctxT_aug[65,s] = V_aug^T @ attnT (row 64 = denominator).
  sc-major tick pipeline: attnV(tick-1) interleaves into scores(tick).
  Normalize via DVE reciprocal + Pool broadcast + fp16 multiply.
  out partial = ctx @ Wo fp16; PSUM evacuated fp16 split ScalarE/DVE
  (halves the out DMA).
"""

import sys

sys.path.insert(0, "/opt/trn_rl_repo")

import numpy as np

import concourse.bacc as bacc
import concourse.mybir as mybir
import concourse.tile as tile
from concourse import bass_utils

N_CORES = 8
B, S, D = 2, 2048, 1024
H_LOC = 4          # heads per core
DH = 64            # head dim
DG = H_LOC * DH    # 256 group dims per core
KC = D // 128      # 8 contraction chunks over D
ST = S // 128      # 16 s/t tiles
SC = S // 512      # 4 512-wide s chunks
MT = DG // 128     # 2 m-tiles of group dims

EW = 576           # exp cols on ScalarE per [128,1024] tile; DVE does the rest
# fp16 Schraudolph: bits = rne(M*raw + B), saturating-uint16, bitcast fp16.
# exp(0.125*raw - 1.5) with sig calibrated offline; e^-1.5 cancels in softmax.
SCH_M = 1024.0 * 1.4427 * 0.125            # 184.67
SCH_B = 1024.0 * (15.0 - 1.4427 * 1.5) - 30.7  # sig=-30.7 (rms-optimal)

F32 = mybir.dt.float32
F16 = mybir.dt.float16
U16 = mybir.dt.uint16


def _build_program(reps=1, num_devices=N_CORES, phases=("proj", "attn", "out")):
    nc = bacc.Bacc("TRN2", target_bir_lowering=False, debug=False,
                   num_devices=num_devices)

    xT_d = nc.dram_tensor("xT", [KC, 128, S], F16, kind="ExternalInput")
    wq_d = nc.dram_tensor("wq", [KC, 128, DG], F16, kind="ExternalInput")
    wk_d = nc.dram_tensor("wk", [KC, 128, DG], F16, kind="ExternalInput")
    wv_d = nc.dram_tensor("wv", [KC, 128, DG], F16, kind="ExternalInput")
    bqk_d = nc.dram_tensor("bqk", [MT, 128, 2], F32, kind="ExternalInput")
    bv_d = nc.dram_tensor("bv", [1, DG], F32, kind="ExternalInput")
    wo_d = nc.dram_tensor("wo", [MT, 128, D], F16, kind="ExternalInput")
    out_d = nc.dram_tensor("out", [S, D], F16, kind="ExternalOutput")

    with tile.TileContext(nc) as tc:
      for _rep in range(reps):
        with (
            tc.tile_pool(name="wpool", bufs=1) as wpool,
            tc.tile_pool(name="mpool", bufs=1) as mpool,
            tc.tile_pool(name="psum", bufs=4, space="PSUM") as pp,
        ):
            # ---- weights / biases (batched DMAs: one per tensor) ----
            wq_t = wpool.tile([128, KC, DG], F16)
            wk_t = wpool.tile([128, KC, DG], F16)
            wv_t = wpool.tile([128, KC, DG], F16)
            wo_t = wpool.tile([128, MT, D], F16)
            bqk_t = wpool.tile([128, MT, 2], F32)
            bv_row = wpool.tile([1, DG], F32)
            bv_bc = wpool.tile([128, DG], F32)
            nbias_t = wpool.tile([128, 1], F32)   # exp bias (-1.5) for ScalarE
            warm_t = wpool.tile([128, 512], F16)
            nc.sync.dma_start(
                wq_t[:], wq_d.ap().rearrange("k p g -> p k g"))
            nc.sync.dma_start(
                wk_t[:], wk_d.ap().rearrange("k p g -> p k g"))
            nc.sync.dma_start(
                wv_t[:], wv_d.ap().rearrange("k p g -> p k g"))
            nc.sync.dma_start(
                wo_t[:], wo_d.ap().rearrange("m p d -> p m d"))
            nc.sync.dma_start(
                bqk_t[:], bqk_d.ap().rearrange("m p c -> p m c"))
            nc.sync.dma_start(bv_row[:], bv_d.ap())
            nc.gpsimd.partition_broadcast(bv_bc[:], bv_row[:])
            nc.gpsimd.memset(nbias_t[:], -1.5)
            nc.gpsimd.memset(warm_t[:], 0.0)
            # PE warmup: keep the HAM activity window busy during input DMA
            ps_w = pp.tile([128, 1024], F32, tag="ps", name="ps_w")
            for _w in range(24):
                nc.tensor.matmul(ps_w[:, 0:512], warm_t[:, 0:128],
                                 warm_t[:], start=(_w == 0), stop=(_w == 23))

            # ---- persistent intermediates ----
            qT_t = mpool.tile([128, MT, S], F16)    # [dg_row, mt, s]
            kT_t = mpool.tile([128, MT, S], F16)
            vaug = mpool.tile([128, ST, H_LOC, DH + 1], F16)
            ctxN = mpool.tile([128, MT, S], F16)    # normalized ctx^T
            nc.gpsimd.memset(vaug[:], 1.0)

            # ---- projections (k-outer for m=0 so PE chases the xT DMA) ----
            with tc.tile_pool(name="xpool", bufs=1) as xpool:
                xT_t = xpool.tile([128, KC, S], F16)
                for k in range(KC):
                    nc.sync.dma_start(xT_t[:, k, :], xT_d.ap()[k])

                for m in (range(MT) if "proj" in phases else []):
                    if m == 0:
                        # k-outer, 4 slots: PE starts on xT chunk 0 while the
                        # rest stream in
                        ps_qks = [
                            pp.tile([128, 1024], F32, tag="ps", name=f"ps_qk{sc}")
                            for sc in range(SC)
                        ]
                        for k in range(KC):
                            for sc in range(SC):
                                sl = slice(sc * 512, sc * 512 + 512)
                                nc.tensor.matmul(
                                    ps_qks[sc][:, 0:512],
                                    wq_t[:, k, 0:128],
                                    xT_t[:, k, sl],
                                    start=(k == 0), stop=(k == KC - 1))
                                nc.tensor.matmul(
                                    ps_qks[sc][:, 512:1024],
                                    wk_t[:, k, 0:128],
                                    xT_t[:, k, sl],
                                    start=(k == 0), stop=(k == KC - 1))
                        for sc in range(SC):
                            sl = slice(sc * 512, sc * 512 + 512)
                            nc.vector.tensor_scalar_add(
                                qT_t[:, m, sl], ps_qks[sc][:, 0:512],
                                bqk_t[:, m, 0:1])
                            nc.vector.tensor_scalar_add(
                                kT_t[:, m, sl], ps_qks[sc][:, 512:1024],
                                bqk_t[:, m, 1:2])
                    else:
                        # sc-outer, 1 slot at a time: frees PSUM early
                        for sc in range(SC):
                            sl = slice(sc * 512, sc * 512 + 512)
                            ps_qk = pp.tile([128, 1024], F32, tag="ps",
                                            name="ps_qk")
                            for k in range(KC):
                                nc.tensor.matmul(
                                    ps_qk[:, 0:512],
                                    wq_t[:, k, m * 128 : m * 128 + 128],
                                    xT_t[:, k, sl],
                                    start=(k == 0), stop=(k == KC - 1))
                                nc.tensor.matmul(
                                    ps_qk[:, 512:1024],
                                    wk_t[:, k, m * 128 : m * 128 + 128],
                                    xT_t[:, k, sl],
                                    start=(k == 0), stop=(k == KC - 1))
                            nc.vector.tensor_scalar_add(
                                qT_t[:, m, sl], ps_qk[:, 0:512], bqk_t[:, m, 0:1])
                            nc.vector.tensor_scalar_add(
                                kT_t[:, m, sl], ps_qk[:, 512:1024], bqk_t[:, m, 1:2])

                # V natural [t, dg], ones column per head
                for st in (range(ST) if "proj" in phases else []):
                    ps_v = pp.tile([128, 1024], F32, tag="ps", name="ps_v")
                    tsl = slice(st * 128, st * 128 + 128)
                    for k in range(KC):
                        nc.tensor.matmul(
                            ps_v[:, 0:DG],
                            xT_t[:, k, tsl],
                            wv_t[:, k, :],
                            start=(k == 0), stop=(k == KC - 1))
                    nc.vector.tensor_add(
                        vaug[:, st, :, 0:DH],
                        ps_v[:, 0:DG].rearrange("p (h d) -> p h d", h=H_LOC),
                        bv_bc[:].rearrange("p (h d) -> p h d", h=H_LOC),
                    )

            # ---- attention: flat (pair, sc) pipeline, V one tick behind ----
            with tc.tile_pool(name="apool", bufs=1) as apool:
              if "attn" in phases:
                ctxu = {
                    (pair, i): apool.tile([65, S], F16, tag=f"ctxu{pair}{i}",
                                          bufs=1, name=f"ctxu{pair}{i}")
                    for pair in range(2) for i in range(2)
                }

                def emit_normalize_chunk(pair, sc):
                    mt = pair
                    ssl = slice(sc * 512, sc * 512 + 512)
                    for i, h in enumerate((2 * pair, 2 * pair + 1)):
                        half = h % 2
                        rs_p = apool.tile([128, 4], F16, tag="rs", bufs=2,
                                          name="rs_p")
                        nc.gpsimd.dma_start(rs_p[:], ctxu[pair, i][64:65, ssl])
                        rr_p = apool.tile([128, 4], F16, tag="rr", bufs=2,
                                          name="rr_p")
                        with nc.allow_low_precision(
                                reason="denominator ~750; fp16 recip 0.05%"):
                            nc.vector.reciprocal(rr_p[:], rs_p[:])
                        r_row = apool.tile([1, 512], F16, tag="rrow", bufs=2,
                                           name="r_row")
                        nc.gpsimd.dma_start(r_row[:], rr_p[:])
                        r_bc = apool.tile([64, 512], F16, tag="rbc", bufs=2,
                                          name="r_bc")
                        nc.gpsimd.partition_broadcast(r_bc[:], r_row[:])
                        if half == 0:
                            nc.vector.tensor_mul(
                                ctxN[0:64, mt, ssl],
                                ctxu[pair, i][0:64, ssl], r_bc[:])
                        else:
                            csh = apool.tile([64, 512], F16, tag="csh", bufs=2,
                                             name="csh")
                            nc.vector.tensor_mul(
                                csh[:], ctxu[pair, i][0:64, ssl], r_bc[:])
                            nc.gpsimd.dma_start(ctxN[64:128, mt, ssl], csh[:])

                def emit_outproj(st):
                    ps_o = pp.tile([128, 1024], F32, tag="ps", name="ps_o")
                    for m in range(MT):
                        for n in range(2):
                            nsl = slice(n * 512, n * 512 + 512)
                            nc.tensor.matmul(
                                ps_o[:, nsl],
                                ctxN[:, m, st * 128 : st * 128 + 128],
                                wo_t[:, m, nsl],
                                start=(m == 0), stop=(m == MT - 1))
                    o4 = emit_outproj.o4
                    if o4 is None:
                        o4 = apool.tile([128, 4, 1024], F16, tag="ot", bufs=2,
                                        name="o4")
                        emit_outproj.o4 = o4
                    nc.scalar.copy(o4[:, st % 4, 0:512], ps_o[:, 0:512])
                    nc.vector.tensor_copy(o4[:, st % 4, 512:1024],
                                          ps_o[:, 512:1024])
                    if st % 4 == 3:
                        st0 = st - 3
                        nc.sync.dma_start(
                            out_d.ap()[st0 * 128 : st0 * 128 + 512, :]
                            .rearrange("(f p) d -> p f d", p=128),
                            o4[:])
                        emit_outproj.o4 = None
                emit_outproj.o4 = None

                ticks = [(p, sc) for p in range(2) for sc in range(SC)]
                prev = None  # (pair, sc, tiles)
                for t in range(len(ticks) + 1):
                    cur_tick = ticks[t] if t < len(ticks) else None
                    ps_c = (pp.tile([128, 1024], F32, tag="ps", name="ps_c")
                            if prev is not None else None)
                    cur = []
                    for st in range(ST):
                        tsl = slice(st * 128, st * 128 + 128)
                        if cur_tick is not None:
                            pair, sc = cur_tick
                            ssl = slice(sc * 512, sc * 512 + 512)
                            ps_s = pp.tile([128, 1024], F32, tag="ps",
                                           name="ps_s")
                            # two K=64 matmuls in disjoint PE row halves
                            nc.tensor.matmul(
                                ps_s[:, 0:512],
                                kT_t[0:64, pair, tsl], qT_t[0:64, pair, ssl])
                            nc.tensor.matmul(
                                ps_s[:, 512:1024],
                                kT_t[64:128, pair, tsl], qT_t[64:128, pair, ssl])
                            at = apool.tile([128, 1024], F16, tag="attnT",
                                            bufs=36, name="at")
                            # engine-split exp: ScalarE [0:EW), DVE Schraudolph
                            nc.scalar.activation(
                                at[:, 0:EW], ps_s[:, 0:EW],
                                mybir.ActivationFunctionType.Exp,
                                bias=nbias_t[:], scale=0.125)
                            nc.vector.tensor_scalar(
                                at[:, EW:1024].bitcast(U16),
                                ps_s[:, EW:1024], SCH_M, SCH_B,
                                mybir.AluOpType.mult, mybir.AluOpType.add)
                            cur.append(at)
                        if prev is not None:
                            vpair, vsc, tiles = prev
                            k = st
                            for i, h in enumerate((2 * vpair, 2 * vpair + 1)):
                                csl = slice(i * 512, i * 512 + 512)
                                nc.tensor.matmul(
                                    ps_c[0:65, csl],
                                    vaug[:, k, h, :],
                                    tiles[k][:, csl],
                                    start=(k == 0), stop=(k == ST - 1))
                    if prev is not None:
                        vpair, vsc, _ = prev
                        psl = slice(vsc * 512, vsc * 512 + 512)
                        nc.scalar.copy(ctxu[vpair, 0][:, psl], ps_c[0:65, 0:512])
                        nc.vector.tensor_copy(
                            ctxu[vpair, 1][:, psl], ps_c[0:65, 512:1024])
                        emit_normalize_chunk(vpair, vsc)
                        if vpair == 1 and "out" in phases:
                            for st_o in range(vsc * 4, vsc * 4 + 4):
                                emit_outproj(st_o)
                    prev = (cur_tick[0], cur_tick[1], cur) if cur_tick else None

    nc.compile()
    return nc


_CACHE = {}


def _get_program():
    if "nc" not in _CACHE:
        _CACHE["nc"] = _build_program()
    return _CACHE["nc"]


def _shard_inputs(x, Wq, bq, Wk, bk, Wv, bv, Wo):
    xT16 = [
        np.ascontiguousarray(x[b].T).astype(np.float16).reshape(KC, 128, S)
        for b in range(B)
    ]
    in_maps = []
    for c in range(N_CORES):
        b, g = c // 4, c % 4
        gs = slice(g * DG, g * DG + DG)
        bqk = np.stack([bq[gs], bk[gs]], axis=-1)  # [DG, 2]
        in_maps.append({
            "xT": xT16[b],
            "wq": np.ascontiguousarray(Wq[:, gs]).astype(np.float16).reshape(KC, 128, DG),
            "wk": np.ascontiguousarray(Wk[:, gs]).astype(np.float16).reshape(KC, 128, DG),
            "wv": np.ascontiguousarray(Wv[:, gs]).astype(np.float16).reshape(KC, 128, DG),
            "bqk": np.ascontiguousarray(bqk).astype(np.float32).reshape(MT, 128, 2),
            "bv": np.ascontiguousarray(bv[gs]).astype(np.float32).reshape(1, DG),
            "wo": np.ascontiguousarray(Wo[gs, :]).astype(np.float16).reshape(MT, 128, D),
        })
    return in_maps


def kernel(x, Wq, bq, Wk, bk, Wv, bv, Wo, bo, _trace=False, _trace_kwargs=None):
    x = np.asarray(x, dtype=np.float32)
    Wq, bq = np.asarray(Wq, np.float32), np.asarray(bq, np.float32)
    Wk, bk = np.asarray(Wk, np.float32), np.asarray(bk, np.float32)
    Wv, bv = np.asarray(Wv, np.float32), np.asarray(bv, np.float32)
    Wo, bo = np.asarray(Wo, np.float32), np.asarray(bo, np.float32)

    nc = _get_program()
    in_maps = _shard_inputs(x, Wq, bq, Wk, bk, Wv, bv, Wo)
    kwargs = {}
    if _trace:
        kwargs["trace"] = True
        kwargs.update(_trace_kwargs or {})
    res = bass_utils.run_bass_kernel_spmd(
        nc, in_maps, core_ids=list(range(N_CORES)), **kwargs)

    out = np.zeros((B, S, D), dtype=np.float32)
    for c in range(N_CORES):
        out[c // 4] += np.asarray(res.results[c]["out"], dtype=np.float32)
    out += bo
    if _trace:
        kernel.last_result = res
    return out
